# revision 31
# baseline (speedup 1.0000x reference)
"""Causal multi-head attention (B=2, S=2048, D=1024, H=16, Dh=64) on 8 TRN2
NeuronCores.

Sharding: core c handles batch c//4 and heads 4*(c%4) .. 4*(c%4)+3 (data
parallel on batch x tensor parallel on heads). Each core is fully
independent: it gets x[b] and the 256-wide column slices of Wq/Wk/Wv for its
4 heads, and returns its heads' outputs as a [2048, 256] slice; the host
reassembles the full [2, 2048, 1024] output by concatenation.

Device kernel (per core); matmul operands in bf16 (fp32 PSUM accumulate),
softmax normalization in fp32:
  A. xT [1024,2048] transposed on host, DMA'd per 128-row k-tile (first
     512-col q-chunk per-k so compute starts ~1us in; later chunks whole).
  B. QT = Wq^T x^T + bq ([c, s] layout, one head pair per 128-partition
     tile), same for KT; V = x Wv + bv in natural [s, c] layout, stored
     augmented with a ones column per head (V_aug[:, 65h+64] = 1,
     prefilled once) so the attention matmul also produces the softmax
     denominator.
  C. Attention as a software-pipelined stream of steps (t, j, tt):
     scores^T[k,q] for both heads of tile t in one 2-bank PSUM tile via a
     concurrent row-tile matmul pair, one Exp per step on ScalarE
     (scale=1/8; scores ~N(0,1), no max subtraction), causal handling by
     column pruning + triangular-mask multiplies (DVE, bf16 2x) on
     diagonal blocks, then O^T[d,q] += V_aug^T @ expS in PSUM (row 64 =
     sum of exp).  The PV for step i is emitted LOOKAHEAD steps after its
     scores, so the PE never head-of-line blocks on the exp latency;
     projection matmuls for chunk j+1 are interleaved between steps at
     single-matmul granularity to keep the PE fed.
  D. Tail per (t, q-chunk, 128-block): PE-transpose the two heads' [65,q]
     PSUM rows back to [q, 65] in one pot tile, one strided reciprocal for
     both denominators, two tensor_scalar muls, DMA out per 128-col half.
"""

import ml_dtypes
import numpy as np

import concourse.bass as bass
import concourse.bass_utils as _bu
import concourse.mybir as mybir
import concourse.tile as tile
from concourse.bass_utils import run_bass_kernel_spmd
from concourse.masks import make_identity, make_upper_triangular

# The default walrus invocation passes --enable-ldw-opt=false, which makes
# every LDWEIGHTS serialize with its matmul (~110ns per matmul here). Flip it.
if not getattr(_bu, "_ldw_patched", False):
    _orig_run_command = _bu.run_command

    def _run_command_ldw(cmd, *a, **kw):
        if isinstance(cmd, (list, tuple)):
            cmd = [
                "--enable-ldw-opt=true" if str(c) == "--enable-ldw-opt=false" else c
                for c in cmd
            ]
        return _orig_run_command(cmd, *a, **kw)

    _bu.run_command = _run_command_ldw
    _bu._ldw_patched = True

# LDW optimization rejects Ldweights instructions that carry semaphore waits;
# this bacc pass is what puts them there (it relocates matmul waits). Our
# matmuls carry at most one wait (_split_sync_waits), so skip the pass.
import concourse.bacc as _bacc

_bacc.Bacc.move_matmul_waits_to_ldweights = lambda self: None

B = 2
S = 2048
D = 1024
H = 16
DH = 64
N_CORES = 8
HPC = 4          # heads per core
CW = HPC * DH    # 256: W column slice width per core
QCH = 512        # q chunk width
F32 = mybir.dt.float32
BF16 = mybir.dt.bfloat16
DT = BF16        # matmul operand dtype
EXP = mybir.ActivationFunctionType.Exp
MULT = mybir.AluOpType.mult
ADD = mybir.AluOpType.add
LOOKAHEAD = 4    # scores run this many steps ahead of their PV

_STATE = {}


def _split_sync_waits(nc, max_waits=1):
    """This walrus rejects instructions carrying more than ~2 sem-waits
    ("Too many sync wait commands"). Move excess waits emitted by Tile onto
    same-engine NoOps inserted right before the instruction."""
    n = 0
    for f in nc.m.functions:
        for bb in f.blocks:
            il = bb.instructions
            i = 0
            while i < len(il):
                ins = il[i]
                si = getattr(ins, "sync_info", None)
                if si is not None and len(si.on_wait) > max_waits:
                    waits = list(si.on_wait)
                    keep = waits[len(waits) - max_waits:]
                    extra = waits[: len(waits) - max_waits]
                    ins.sync_info = mybir.SyncInfo(
                        on_wait=keep, on_update=list(si.on_update)
                    )
                    pos = i
                    for j in range(0, len(extra), max_waits):
                        nop = mybir.InstNoOp(
                            name=f"{ins.name}-waitsplit{j}",
                            engine=ins.engine,
                            sync_info=mybir.SyncInfo(
                                on_wait=extra[j : j + max_waits], on_update=[]
                            ),
                            bass_nofuse=True,
                        )
                        il.insert(pos, nop)
                        pos += 1
                        i += 1
                    n += 1
                i += 1
    return n


def _strip_ldweights_sync(nc):
    """walrus's LDW optimization (--enable-ldw-opt=true) rejects explicit
    Ldweights instructions. Delete the companions (their sync moves onto
    NoOps) and mark the matmuls self-loading so walrus generates and
    optimizes the weight loads itself."""
    n = 0
    for f in nc.m.functions:
        for bb in f.blocks:
            il = bb.instructions
            out = []
            for ins in il:
                if isinstance(ins, mybir.InstLdweights):
                    si = getattr(ins, "sync_info", None)
                    if si is not None and si.on_wait:
                        out.append(
                            mybir.InstNoOp(
                                name=f"{ins.name}-ldwait",
                                engine=ins.engine,
                                sync_info=mybir.SyncInfo(
                                    on_wait=list(si.on_wait), on_update=[]
                                ),
                                bass_nofuse=True,
                            )
                        )
                    if si is not None and si.on_update:
                        out.append(
                            mybir.InstNoOp(
                                name=f"{ins.name}-ldupd",
                                engine=ins.engine,
                                sync_info=mybir.SyncInfo(
                                    on_wait=[], on_update=list(si.on_update)
                                ),
                                bass_nofuse=True,
                            )
                        )
                    n += 1
                    continue
                if isinstance(ins, mybir.InstMatmult) and ins.ldweights is False:
                    ins.ldweights = None
                out.append(ins)
            il[:] = out
    return n


def _build():
    nc = bass.Bass()
    ND_ = D // 128
    # all inputs host-packed partition-major: [p, k*inner+i] = src[k*128+p, i],
    # so every DMA row is a multi-KB contiguous run
    xt_d = nc.dram_tensor("xt", [128, ND_ * S], BF16, kind="ExternalInput")
    wq_d = nc.dram_tensor("wq", [128, ND_ * CW], BF16, kind="ExternalInput")
    wk_d = nc.dram_tensor("wk", [128, ND_ * CW], BF16, kind="ExternalInput")
    wv_d = nc.dram_tensor("wv", [128, ND_ * CW], BF16, kind="ExternalInput")
    bq_d = nc.dram_tensor("bq", [CW], F32, kind="ExternalInput")
    bk_d = nc.dram_tensor("bk", [CW], F32, kind="ExternalInput")
    bv_d = nc.dram_tensor("bv", [CW], BF16, kind="ExternalInput")
    out_d = nc.dram_tensor("out", [S, CW], F32, kind="ExternalOutput")

    ND = D // 128   # 8 d tiles
    NS = S // 128   # 16 s tiles
    NQ = S // QCH   # 4 q chunks

    with tile.TileContext(nc) as tc:
        with (
            tc.tile_pool(name="const", bufs=1) as cp,
            tc.tile_pool(name="big", bufs=1) as bigp,
        ):
            xTall = bigp.tile([128, ND * S], DT, tag="xTall")
            xT = [xTall[:, S * k : S * (k + 1)] for k in range(ND)]
            xt3 = xt_d.rearrange("p (k s) -> p k s", s=S)
            xTall3 = xTall.rearrange("p (k s) -> p k s", s=S)

            wqall = bigp.tile([128, ND * CW], DT, tag="wqall")
            wkall = bigp.tile([128, ND * CW], DT, tag="wkall")
            wvall = bigp.tile([128, ND * CW], DT, tag="wvall")
            wq = [wqall[:, CW * k : CW * (k + 1)] for k in range(ND)]
            wk = [wkall[:, CW * k : CW * (k + 1)] for k in range(ND)]
            wv = [wvall[:, CW * k : CW * (k + 1)] for k in range(ND)]

            bqs = cp.tile([128, 2], F32, tag="bqs")
            bks = cp.tile([128, 2], F32, tag="bks")
            bvr = cp.tile([1, CW], DT, tag="bvr")
            nc.sync.dma_start(out=bqs[:], in_=bq_d.rearrange("(t p) -> p t", p=128))
            nc.sync.dma_start(out=bks[:], in_=bk_d.rearrange("(t p) -> p t", p=128))
            nc.sync.dma_start(out=bvr[:], in_=bv_d[None, :])

            # contiguous 4KB-row DMAs; round-robin the issuing engine so
            # several DMA rings stream in parallel from t=0
            rings = [nc.sync, nc.gpsimd, nc.scalar]
            rings[0].dma_start(out=wqall[:], in_=wq_d[:, :])
            rings[1].dma_start(out=wkall[:], in_=wk_d[:, :])
            rings[2].dma_start(out=wvall[:], in_=wv_d[:, :])
            for k in range(ND):
                rings[k % 3].dma_start(out=xTall3[:, k, :], in_=xt3[:, k, :])

            idf = cp.tile([128, 128], F32, tag="idf")
            idfb = cp.tile([128, 128], DT, tag="idfb")
            tri32 = cp.tile([128, 128], F32, tag="tri32")
            ones32 = cp.tile([128, 128], F32, tag="ones32")
            tri = cp.tile([128, 128], DT, tag="tri")
            onesb = cp.tile([1, 128], DT, tag="onesb")
            ones_d = cp.tile([128, HPC], DT, tag="ones_d")

            def emit_consts():
                make_identity(nc, idf[:])
                make_upper_triangular(nc, tri32[:], val=1.0, diag=True)
                nc.gpsimd.memset(ones32[:], 1.0)
                nc.vector.tensor_copy(idfb[:], idf[:])
                nc.vector.tensor_copy(tri[:], tri32[:])
                nc.vector.tensor_copy(onesb[:], ones32[0:1, :])
                nc.vector.tensor_copy(ones_d[:], ones32[:, 0:HPC])

            qt = [bigp.tile([128, S], DT, tag=f"qt{t}", name=f"qt{t}") for t in range(2)]
            kt = [bigp.tile([128, S], DT, tag=f"kt{t}", name=f"kt{t}") for t in range(2)]
            va = [bigp.tile([128, 65 * HPC], DT, tag=f"va{i}", name=f"va{i}") for i in range(NS)]

            with (
                tc.tile_pool(name="esp", bufs=6) as esp,
                tc.tile_pool(name="otp", bufs=4) as otp,
                tc.tile_pool(name="tlp", bufs=4) as tlp,
                tc.tile_pool(name="onp", bufs=4) as onp,
                tc.tile_pool(name="pp", bufs=2, space="PSUM") as pp,
                tc.tile_pool(name="psc", bufs=2, space="PSUM") as psc,
                tc.tile_pool(name="pso", bufs=2, space="PSUM") as pso,
            ):
                bcv = cp.tile([128, CW], F32, tag="bcv")

                def emit_consts2():
                    # ones column of V_aug: prefill once
                    for i in range(NS):
                        nc.vector.tensor_copy(
                            va[i].rearrange("p (h e) -> p h e", h=HPC)[:, :, DH : DH + 1],
                            ones_d[:, :, None],
                        )
                    # V-bias broadcast tile
                    ppb = pp.tile([128, CW], F32, tag="ppt")
                    nc.tensor.matmul(
                        ppb[:], onesb[0:1, :], bvr[0:1, :], start=True, stop=True
                    )
                    nc.vector.tensor_copy(bcv[:], ppb[:])

                # ---- projection micro-tasks (single matmuls + finishers) ----
                def qk_minis(w, dstT, bsl, t, j, label, pool=None):
                    q0 = QCH * j
                    ppt = [None]
                    pool = pool or pp

                    def mini(k):
                        def run():
                            if k == 0:
                                tag = "pss" if pool is psc else "ppt"
                                ppt[0] = pool.tile(
                                    [128, QCH], F32, tag=tag,
                                    name=f"ppt{label}{t}{j}",
                                )
                            nc.tensor.matmul(
                                ppt[0][:, 0:QCH],
                                w[k][:, 128 * t : 128 * (t + 1)],
                                xT[k][:, q0 : q0 + QCH],
                                start=(k == 0),
                                stop=(k == ND - 1),
                            )

                        return run

                    def fin():
                        nc.vector.tensor_scalar_add(
                            dstT[t][:, q0 : q0 + QCH], ppt[0][:, 0:QCH],
                            bsl[:, t : t + 1],
                        )

                    return [mini(k) for k in range(ND)] + [fin]

                def v_minis(i, pool=None):
                    ppv = [None]
                    pool = pool or pp

                    def mini(k):
                        def run():
                            if k == 0:
                                tag = "po" if pool is pso else "ppt"
                                ppv[0] = pool.tile(
                                    [128, CW], F32, tag=tag, name=f"ppv{i}"
                                )
                            nc.tensor.matmul(
                                ppv[0][:, 0:CW],
                                xT[k][:, 128 * i : 128 * (i + 1)],
                                wv[k],
                                start=(k == 0),
                                stop=(k == ND - 1),
                            )

                        return run

                    def fin():
                        nc.vector.tensor_tensor(
                            out=va[i].rearrange("p (h e) -> p h e", h=HPC)[:, :, 0:DH],
                            in0=ppv[0][:, 0:CW].rearrange("p (h e) -> p h e", e=DH),
                            in1=bcv.rearrange("p (h e) -> p h e", e=DH),
                            op=ADD,
                        )

                    return [mini(k) for k in range(ND)] + [fin]

                def proj_fillers(j):
                    tasks = []
                    for w, dstT, bsl, lbl in ((wq, qt, bqs, "q"), (wk, kt, bks, "k")):
                        for t in range(2):
                            tasks.extend(qk_minis(w, dstT, bsl, t, j, lbl))
                    for i in range(HPC * j, HPC * j + HPC):
                        tasks.extend(v_minis(i))
                    return tasks

                # ---- attention steps ----
                def make_step(t, j, tt, last):
                    q0 = QCH * j
                    r = max(0, 128 * tt - q0)
                    box = {}

                    def sc():
                        pss = psc.tile([128, 2 * QCH], F32, tag="pss")
                        for h, base in ((0, 0), (1, 64)):
                            nc.tensor.matmul(
                                pss[:, h * QCH + r : (h + 1) * QCH],
                                kt[t][base : base + 64, 128 * tt : 128 * (tt + 1)],
                                qt[t][base : base + 64, q0 + r : q0 + QCH],
                                start=True,
                                stop=True,
                                tile_position=(base, 0),
                            )
                        es = esp.tile([128, 2 * QCH], DT, tag="es")
                        if r == 0:
                            nc.scalar.activation(es[:], pss[:], EXP, scale=0.125)
                        else:
                            nc.scalar.activation(
                                es.rearrange("p (h q) -> p h q", h=2)[:, :, r:QCH],
                                pss.rearrange("p (h q) -> p h q", h=2)[:, :, r:QCH],
                                EXP,
                                scale=0.125,
                            )
                        if tt >= HPC * j:  # diagonal block: triangular mask
                            for h in range(2):
                                nc.vector.tensor_tensor(
                                    out=es[:, h * QCH + r : h * QCH + r + 128],
                                    in0=es[:, h * QCH + r : h * QCH + r + 128],
                                    in1=tri[:],
                                    op=MULT,
                                )
                        box["es"] = es

                    def pv(po):
                        es = box["es"]
                        for h in range(2):
                            hcol = 65 * (2 * t + h)
                            nc.tensor.matmul(
                                po[h][0:65, r:QCH],
                                va[tt][:, hcol : hcol + 65],
                                es[:, h * QCH + r : (h + 1) * QCH],
                                start=(tt == 0),
                                stop=(tt == last),
                            )

                    return sc, pv

                def tail_tasks(t, j, ots):
                    q0 = QCH * j
                    # the kernel's serial epilogue: exp is finished, so route
                    # the last tail's element-wise work to the scalar engine
                    on_act = t == 1 and j == NQ - 1

                    def one(c):
                        def run():
                            # 66-wide per head: bf16 PSUM writes must be
                            # 4-byte aligned
                            pot = pp.tile([128, 132], DT, tag="ppt", name=f"pot{t}{j}{c}")
                            for h in range(2):
                                nc.tensor.transpose(
                                    pot[:, 66 * h : 66 * h + 65],
                                    ots[h][0:65, 128 * c : 128 * (c + 1)],
                                    idfb[0:65, 0:65],
                                )
                            rc = tlp.tile([128, 2], F32, tag="rc")
                            nc.vector.reciprocal(
                                rc[:],
                                pot.rearrange("p (h e) -> p h e", e=66)[:, :, 64],
                            )
                            on = onp.tile([128, 128], F32, tag="on")
                            for h in range(2):
                                eng_mul = (
                                    (lambda o, i_, s: nc.scalar.mul(o, i_, s))
                                    if on_act
                                    else nc.vector.tensor_scalar_mul
                                )
                                eng_mul(
                                    on[:, DH * h : DH * (h + 1)],
                                    pot[:, 66 * h : 66 * h + DH],
                                    rc[:, h : h + 1],
                                )
                            nc.sync.dma_start(
                                out=out_d[
                                    q0 + 128 * c : q0 + 128 * (c + 1),
                                    128 * t : 128 * (t + 1),
                                ],
                                in_=on[:],
                            )

                        return run

                    return [one(c) for c in range(QCH // 128)]

                # ---- emission: startup ----
                # consts first (no x dependency), then six chunk-0 chains
                # interleaved per k-tile so the PE tracks the x DMA stream:
                # Q pair in pp, K pair borrowing psc, V(i=0,1) borrowing pso
                emit_consts()
                emit_consts2()
                chains6 = (
                    [qk_minis(wq, qt, bqs, t, 0, "q") for t in range(2)]
                    + [qk_minis(wk, kt, bks, t, 0, "k", pool=psc) for t in range(2)]
                    + [v_minis(i, pool=pso) for i in range(2)]
                )
                for k in range(ND):
                    for ch in chains6:
                        ch[k]()
                for ch in chains6:
                    ch[ND]()
                for i in range(2, HPC):
                    for task in v_minis(i, pool=pso):
                        task()

                pvq = []      # pv closures (already bound to their po)
                fillers = []

                def pop_filler(nmax):
                    for _ in range(min(nmax, len(fillers))):
                        fillers.pop(0)()

                def pop_pv():
                    pvq.pop(0)()

                def finish_t(t, j, po):
                    # runs inside the last PV closure of (t, j): free the po
                    # PSUM tiles via ot copies, then queue the tail work
                    ots = {}
                    for h in range(2):
                        ot = otp.tile([128, QCH], DT, tag="ot", name=f"ot{t}{j}{h}")
                        if t == 1 and j == NQ - 1:
                            nc.scalar.copy(ot[0:65, :], po[h][0:65, :])
                        else:
                            nc.vector.tensor_copy(ot[0:65, :], po[h][0:65, :])
                        ots[h] = ot
                    fillers.extend(tail_tasks(t, j, ots))

                carry = [0.0]

                def pace(rem_steps):
                    if rem_steps > 0 and fillers:
                        carry[0] += len(fillers) / (rem_steps + 1)
                        n = min(int(carry[0]), 5)
                        if n > 0:
                            carry[0] -= n
                            pop_filler(n)

                total_steps = sum(2 * (HPC * j + HPC) for j in range(NQ))
                done = 0
                for j in range(NQ):
                    # hard fillers for next chunk (must be fully emitted
                    # before that chunk's scores/PVs read qt/kt/va)
                    fillers.extend(proj_fillers(j + 1) if j + 1 < NQ else [])
                    for t in range(2):
                        last = HPC * j + HPC - 1
                        po = [
                            pso.tile([128, QCH], F32, tag="po", name=f"po{t}{j}{h}")
                            for h in range(2)
                        ]
                        for tt in range(last + 1):
                            sc, pv = make_step(t, j, tt, last)
                            if tt == last:
                                def pv_last(pv=pv, po=po, t=t, j=j):
                                    pv(po)
                                    finish_t(t, j, po)
                                pvq.append(pv_last)
                            else:
                                pvq.append(lambda pv=pv, po=po: pv(po))
                            sc()
                            if len(pvq) > LOOKAHEAD:
                                pop_pv()
                            done += 1
                            pace(total_steps - done)
                    # chunk boundary: flush remaining fillers
                    pop_filler(len(fillers))
                while pvq:
                    pop_pv()
                pop_filler(len(fillers))

    _strip_ldweights_sync(nc)
    _split_sync_waits(nc)
    return nc


def _get_nc():
    if "nc" not in _STATE:
        _STATE["nc"] = _build()
    return _STATE["nc"]


def kernel(**inputs):
    x = np.asarray(inputs["x"], dtype=np.float32)
    wq = np.asarray(inputs["Wq"], dtype=np.float32).astype(ml_dtypes.bfloat16)
    wk = np.asarray(inputs["Wk"], dtype=np.float32).astype(ml_dtypes.bfloat16)
    wv = np.asarray(inputs["Wv"], dtype=np.float32).astype(ml_dtypes.bfloat16)
    bq = np.asarray(inputs["bq"], dtype=np.float32)
    bk = np.asarray(inputs["bk"], dtype=np.float32)
    bv = np.asarray(inputs["bv"], dtype=np.float32).astype(ml_dtypes.bfloat16)

    ND_ = D // 128

    def pack(a):  # [D, inner] -> [128, ND*inner], row k*128+p -> [p, k*inner:]
        inner = a.shape[1]
        return np.ascontiguousarray(
            a.reshape(ND_, 128, inner).transpose(1, 0, 2).reshape(128, ND_ * inner)
        )

    xts = [pack(x[b].T.astype(ml_dtypes.bfloat16)) for b in range(B)]

    in_maps = []
    for c in range(N_CORES):
        b, hg = divmod(c, HPC)
        sl = slice(CW * hg, CW * (hg + 1))
        in_maps.append(
            {
                "xt": xts[b],
                "wq": pack(np.ascontiguousarray(wq[:, sl])),
                "wk": pack(np.ascontiguousarray(wk[:, sl])),
                "wv": pack(np.ascontiguousarray(wv[:, sl])),
                "bq": np.ascontiguousarray(bq[sl]),
                "bk": np.ascontiguousarray(bk[sl]),
                "bv": np.ascontiguousarray(bv[sl]),
            }
        )

    nc = _get_nc()
    res = run_bass_kernel_spmd(nc, in_maps, list(range(N_CORES)))
    _STATE["last_result"] = res

    out = np.empty((B, S, D), dtype=np.float32)
    for c in range(N_CORES):
        b, hg = divmod(c, HPC)
        out[b, :, CW * hg : CW * (hg + 1)] = res.results[c]["out"]
    return out
